# revision 13
# baseline (speedup 1.0000x reference)
"""Distributed 2-layer GATConv + per-subgraph Linear on 8 TRN2 NeuronCores.

Key identity: out_dst = (sum_e ex_e * x_src_e) @ W / sum_e ex_e + b, i.e. the
dense transform W is applied AFTER aggregation (per node), and the attention
logits al_s/al_d are per-node scalars x . (W @ a). So per edge we only ship
the raw src feature row plus two scalars; no per-edge transform matmul.

Launches:
  A1: per-node al_s1/al_d1 = x . (W1@a_src1), x . (W1@a_dst1)   (tiny)
  C1: layer-1 edge aggregation + W1/bias/relu epilogue + per-node
      al_s2/al_d2 for layer 2 (fused)
  C2: layer-2 edge aggregation + W2/bias/relu epilogue
  F : per-subgraph Linear as a PE matvec

Edge layout: dst-sharded edges, sorted by dst, bin-packed into 32-dst
subtiles of 256 slots (2 chunks of 128). Per chunk one PE matmul:
lhsT = [x_src | 1] (128 edges x 65, stationary), rhs = onehot(dstcol)*ex
(128 x 32) accumulating [sum ex*x | sum ex] per dst column into PSUM.
A second small matmul per 4 subtiles applies Wext = [[W,0],[b,1]] giving
[num@W + denom*b | denom] per dst; epilogue multiplies by 1/denom (relu'd).
Device outputs stay in SBUF-native [128, cols] layouts (large DMA
descriptors); the host unswizzles by precomputed index maps. The host
performs only data movement; model arithmetic runs on device.
"""

import dataclasses
import math
import numpy as np

import concourse.bass as bass
import concourse.bacc as bacc
import concourse.mybir as mybir
import concourse.tile as tile
from concourse.bass_utils import run_bass_kernel_spmd

F32 = mybir.dt.float32
BF16 = mybir.dt.bfloat16
NP_BF16 = mybir.dt.np(BF16)

NCORES = 8
F = 64
SUBG = 20
XW = 65           # src feature cols + ones col
SUB_SLOTS = 256   # slots per subtile (2 chunks)
SUB_CAP = 16
NEG_SLOPE = 0.2
WCH = 64          # chunks per window


@dataclasses.dataclass
class Cfg:
    n: int
    npc: int
    nsub: int
    e_pad: int
    ct: int
    nw: int
    ng: int


def make_cfg(n_nodes: int, n_edges: int, bump: int = 0) -> Cfg:
    npc = n_nodes // NCORES
    epc = (n_edges + n_nodes) / NCORES
    slots = epc * 1.09 + 2 * SUB_SLOTS
    nsub = math.ceil(slots / SUB_SLOTS / (2 * WCH // 2)) * (WCH // 2) * 2
    nsub += (WCH // 2) * 2 * bump
    e_pad = nsub * SUB_SLOTS
    ct = e_pad // 128
    return Cfg(n=n_nodes, npc=npc, nsub=nsub, e_pad=e_pad,
               ct=ct, nw=ct // WCH, ng=npc // SUBG)


# ---------------------------------------------------------------- host prep

def _pack_bins(deg, cfg):
    """Largest-fit packing of dsts into bins of 256 slots / 32 dsts."""
    npc = deg.shape[0]
    vmax = int(deg.max())
    assert vmax <= SUB_SLOTS
    order = np.argsort(deg, kind="stable")
    bucket_start = np.searchsorted(deg[order], np.arange(vmax + 2))
    ptr = bucket_start[:-1].copy()
    cnt = (bucket_start[1:] - bucket_start[:-1]).copy()
    sub_of = np.empty(npc, np.int32)
    col_of = np.empty(npc, np.int32)
    remaining = npc
    b = 0
    while remaining > 0:
        rem = SUB_SLOTS
        col = 0
        while col < SUB_CAP:
            v = min(rem, vmax)
            while v >= 0 and cnt[v] == 0:
                v -= 1
            if v < 0:
                break
            d = order[ptr[v]]
            ptr[v] += 1
            cnt[v] -= 1
            sub_of[d] = b
            col_of[d] = col
            col += 1
            rem -= v
            remaining -= 1
        b += 1
        if b > cfg.nsub:
            raise OverflowError("bin packing exceeded subtile budget")
    return sub_of, col_of


def _prep_core(src, dst_local, cfg):
    """Per-core slot layout. Returns (slot_src, slot_dst, seg_slin,
    stg_of_node)."""
    npc, e_pad = cfg.npc, cfg.e_pad
    deg = np.bincount(dst_local, minlength=npc)
    sub_of, col_of = _pack_bins(deg, cfg)

    order = np.argsort(dst_local, kind="stable")
    src_bd = src[order]
    starts = np.zeros(npc + 1, np.int64)
    np.cumsum(deg, out=starts[1:])

    binkey = sub_of.astype(np.int64) * SUB_CAP + col_of
    dorder = np.argsort(binkey, kind="stable")
    deg_bo = deg[dorder]
    csum = np.cumsum(deg_bo)
    bin_ids = sub_of[dorder]
    nb = int(bin_ids.max()) + 1
    first_of_bin = np.searchsorted(bin_ids, np.arange(nb), side="left")
    base_cum = np.where(first_of_bin > 0, csum[np.maximum(first_of_bin - 1, 0)], 0)
    off_in_bin = np.zeros(npc, np.int64)
    off_in_bin[dorder] = (csum - deg_bo) - base_cum[bin_ids]
    slot0 = sub_of.astype(np.int64) * SUB_SLOTS + off_in_bin

    slot_src = np.zeros(e_pad, np.int64)
    slot_dst = np.zeros(e_pad, np.int64)
    seg_slin = np.full(e_pad, 64, np.int64)

    edst = dst_local[order]
    within = np.arange(len(order), dtype=np.int64) - starts[edst]
    eslot = slot0[edst] + within
    slot_src[eslot] = src_bd
    slot_dst[eslot] = edst
    seg_slin[eslot] = col_of[edst]
    stg_of_node = sub_of.astype(np.int64) * SUB_CAP + col_of
    return slot_src, slot_dst, seg_slin, stg_of_node


def bc(ap, ins_idx, pair):
    aps = list(ap.ap)
    aps.insert(ins_idx, list(pair))
    return dataclasses.replace(ap, ap=aps)


def rep(ap, offset, new_ap):
    return dataclasses.replace(ap, offset=ap.offset + offset, ap=new_ap)


# ---------------------------------------------------------------- A1 launch

def _build_node_al(nc, cfg):
    npc = cfg.npc
    nt = npc // 128
    xfm_p = nc.declare_dram_parameter("xfm", [F, npc], BF16, isOutput=False)
    v12_p = nc.declare_dram_parameter("v12", [F, 2], BF16, isOutput=False)
    # SBUF-native: alsd[p, 2t+k] = al_{s,d}[node t*128+p]
    out_p = nc.declare_dram_parameter("alsd", [128, nt * 2], F32,
                                      isOutput=True)
    with nc.psum_tensor([128, 4096], F32) as PS, tile.TileContext(nc) as tc:
        with tc.tile_pool(name="p", bufs=1) as pool:
            xfm_s = pool.tile([F, npc], BF16, name="xfm_s")
            nc.sync.dma_start(xfm_s[:], xfm_p[:])
            v12_s = pool.tile([F, 2], BF16, name="v12_s")
            nc.sync.dma_start(v12_s[:], v12_p[:])
            acc = pool.tile([128, nt * 2], F32, name="acc")
            for t in range(nt):
                bank = (t // 256) % 8
                ps = rep(PS[:], bank * 512 + (t % 256) * 2,
                         [[4096, 128], [1, 2]])
                nc.tensor.matmul(ps, xfm_s[:, t * 128:(t + 1) * 128],
                                 v12_s[:], start=True, stop=True)
            nbank = (nt + 255) // 256
            for b in range(nbank):
                n = min(256, nt - b * 256)
                psb = rep(PS[:], (b % 8) * 512, [[4096, 128], [1, 2 * n]])
                nc.vector.tensor_copy(acc[:, b * 512:b * 512 + 2 * n], psb)
            nc.sync.dma_start(out_p[:], acc[:])


# ---------------------------------------------------------------- GAT launch

def _build_gat(nc, cfg, first):
    AT = mybir.ActivationFunctionType
    OP = mybir.AluOpType
    ct, nw = cfg.ct, cfg.nw

    xe_p = nc.declare_dram_parameter("xe", [128, ct * XW], BF16, isOutput=False)
    esc_p = nc.declare_dram_parameter("esc", [128, ct * 2], BF16,
                                      isOutput=False)
    seg_p = nc.declare_dram_parameter("seg", [128, ct], BF16, isOutput=False)
    wext_p = nc.declare_dram_parameter("wext", [128, XW], BF16, isOutput=False)
    iota_p = nc.declare_dram_parameter("iotar", [128, SUB_CAP * WCH], BF16,
                                       isOutput=False)
    # SBUF-native: stg[p, (w*4+g)*64 + f] = h[slot (w*4+g)*128+p, f]
    stg_p = nc.declare_dram_parameter("stg", [128, nw * 256], BF16,
                                      isOutput=True)
    if first:
        vs_p = nc.declare_dram_parameter("vsrep", [128, F], BF16,
                                         isOutput=False)
        vd_p = nc.declare_dram_parameter("vdrep", [128, F], BF16,
                                         isOutput=False)
        # al2[p, k*nw*4 + w*4 + g] = al2_k[slot (w*4+g)*128+p]
        al2_p = nc.declare_dram_parameter("al2", [128, 2 * nw * 4], F32,
                                          isOutput=True)

    with nc.psum_tensor([128, 4096], F32) as PS, tile.TileContext(nc) as tc:
        with (
            tc.tile_pool(name="const", bufs=1) as cpool,
            tc.tile_pool(name="xw", bufs=2) as xpool,
            tc.tile_pool(name="esc", bufs=2) as epool,
            tc.tile_pool(name="sw", bufs=2) as swpool,
            tc.tile_pool(name="u", bufs=2) as upool,
            tc.tile_pool(name="stage", bufs=2) as stpool,
            tc.tile_pool(name="small", bufs=3) as smpool,
        ):
            wext_s = cpool.tile([128, XW], BF16, name="wext_s")
            nc.sync.dma_start(wext_s[:], wext_p[:])
            iota_s = cpool.tile([128, SUB_CAP * WCH], BF16, name="iota_s")
            nc.sync.dma_start(iota_s[:], iota_p[:])
            if first:
                vs_s = cpool.tile([128, F], BF16, name="vs_s")
                nc.sync.dma_start(vs_s[:], vs_p[:])
                vd_s = cpool.tile([128, F], BF16, name="vd_s")
                nc.sync.dma_start(vd_s[:], vd_p[:])
                al2acc = cpool.tile([128, 2 * nw * 4], F32, name="al2acc")

            pend = None

            def emit_tail(pw, u_prev):
                base2 = (2 + (pw % 2)) * 512
                for g in range(4):
                    ps2 = rep(PS[:], base2 + g * 128, [[4096, 128], [1, XW]])
                    nc.tensor.matmul(ps2, u_prev[:, g * 128:(g + 1) * 128],
                                     wext_s[0:XW, :], start=True, stop=True)
                dsb = smpool.tile([128, 4], F32, name="dsb", tag="dsb")
                nc.vector.tensor_scalar(
                    dsb[:], rep(PS[:], base2 + F, [[4096, 128], [128, 4]]),
                    1e-20, None, op0=OP.add)
                rd = smpool.tile([128, 4], F32, name="rd", tag="rd")
                nc.vector.reciprocal(rd[:], dsb[:])
                stg_t = stpool.tile([128, 4 * F], BF16, name="stg", tag="stg")
                stg3 = stg_t[:].rearrange("p (g e) -> p g e", e=F)
                ps2num = rep(PS[:], base2, [[4096, 128], [128, 4], [1, F]])
                nc.vector.tensor_tensor(stg3, ps2num, bc(rd[:], 2, (0, F)),
                                        op=OP.mult)
                nc.vector.tensor_scalar(stg_t[:], stg_t[:], 0.0, None,
                                        op0=OP.max)
                nc.sync.dma_start(stg_p[:, pw * 256:(pw + 1) * 256], stg_t[:])
                if first:
                    tmp = stpool.tile([128, 4 * F], BF16, name="tmp",
                                      tag="tmp")
                    tmp3 = tmp[:].rearrange("p (g e) -> p g e", e=F)
                    nc.gpsimd.tensor_tensor(tmp3, stg3, bc(vs_s[:], 1, (0, 4)),
                                            op=OP.mult)
                    o0 = rep(al2acc[:], pw * 4, [[2 * nw * 4, 128], [1, 4]])
                    nc.vector.tensor_reduce(o0, tmp3, mybir.AxisListType.X,
                                            OP.add)
                    tmp2 = stpool.tile([128, 4 * F], BF16, name="tmp2",
                                       tag="tmp2")
                    tmp23 = tmp2[:].rearrange("p (g e) -> p g e", e=F)
                    nc.gpsimd.tensor_tensor(tmp23, stg3,
                                            bc(vd_s[:], 1, (0, 4)),
                                            op=OP.mult)
                    o1 = rep(al2acc[:], nw * 4 + pw * 4,
                             [[2 * nw * 4, 128], [1, 4]])
                    nc.vector.tensor_reduce(o1, tmp23, mybir.AxisListType.X,
                                            OP.add)

            for w in range(nw):
                xw_t = xpool.tile([128, WCH * XW], BF16, name="xw", tag="xw")
                nc.sync.dma_start(xw_t[:], xe_p[:, w * WCH * XW:
                                                (w + 1) * WCH * XW])
                esc_t = epool.tile([128, WCH * 2], BF16, name="esc", tag="esc")
                nc.sync.dma_start(esc_t[:], esc_p[:, w * WCH * 2:
                                                  (w + 1) * WCH * 2])
                seg_t = epool.tile([128, WCH], BF16, name="seg", tag="seg")
                nc.sync.dma_start(seg_t[:], seg_p[:, w * WCH:(w + 1) * WCH])
                als_v = rep(esc_t[:], 0, [[WCH * 2, 128], [2, WCH]])
                ald_v = rep(esc_t[:], 1, [[WCH * 2, 128], [2, WCH]])

                e_t = smpool.tile([128, WCH], F32, name="e", tag="e")
                nc.vector.tensor_tensor(e_t[:], als_v, ald_v, op=OP.add)
                lr_t = smpool.tile([128, WCH], F32, name="lr", tag="lr")
                nc.vector.tensor_scalar(lr_t[:], e_t[:], NEG_SLOPE, None,
                                        op0=OP.mult)
                nc.vector.tensor_tensor(lr_t[:], lr_t[:], e_t[:], op=OP.max)
                exb = smpool.tile([128, WCH], BF16, name="exb", tag="exb")
                nc.scalar.activation(exb[:], lr_t[:], AT.Exp)

                # Sw[p, d, c]: dst-col-major so every operand is packed
                SC = SUB_CAP
                Sw_t = swpool.tile([128, SC * WCH], BF16, name="Sw", tag="Sw")
                sw3 = rep(Sw_t[:], 0, [[SC * WCH, 128], [WCH, SC], [1, WCH]])
                io3 = rep(iota_s[:], 0, [[SC * WCH, 128], [WCH, SC], [1, WCH]])
                nc.vector.tensor_tensor(sw3, io3, bc(seg_t[:], 1, (0, SC)),
                                        op=OP.is_equal)
                nc.vector.tensor_tensor(sw3, sw3, bc(exb[:], 1, (0, SC)),
                                        op=OP.mult)

                u_t = upool.tile([XW, 512], BF16, name="u", tag="u")
                for cl in range(WCH):
                    sb, k = divmod(cl, 2)
                    bank = w % 2
                    psu = rep(PS[:], bank * 512 + sb * SC,
                              [[4096, XW], [1, SC]])
                    swv = rep(Sw_t[:], cl, [[SC * WCH, 128], [WCH, SC]])
                    nc.tensor.matmul(psu, xw_t[:, cl * XW:(cl + 1) * XW],
                                     swv, start=(k == 0), stop=(k == 1))
                    if cl == 15 and pend is not None:
                        emit_tail(*pend)
                        pend = None
                    if cl == 63:
                        psb = rep(PS[:], bank * 512, [[4096, XW], [1, 512]])
                        nc.vector.tensor_copy(u_t[:], psb)
                        pend = (w, u_t)
            emit_tail(*pend)
            if first:
                nc.sync.dma_start(al2_p[:], al2acc[:])


# ---------------------------------------------------------------- final linear

def _build_final(nc, cfg):
    OP = mybir.AluOpType
    ng = cfg.ng
    nk = SUBG * F // 128  # 10
    hk_p = nc.declare_dram_parameter("hk", [128, nk * ng], BF16,
                                     isOutput=False)
    wout_p = nc.declare_dram_parameter("woutk", [128, nk], BF16,
                                       isOutput=False)
    bout_p = nc.declare_dram_parameter("boutr", [1, 1], F32, isOutput=False)
    out_p = nc.declare_dram_parameter("out", [ng, 1], F32, isOutput=True)
    with nc.psum_tensor([128, 4096], F32) as PS, tile.TileContext(nc) as tc:
        with tc.tile_pool(name="p", bufs=1) as pool:
            hk_s = pool.tile([128, nk * ng], BF16, name="hk_s")
            nc.sync.dma_start(hk_s[:], hk_p[:])
            wout_s = pool.tile([128, nk], BF16, name="wout_s")
            nc.sync.dma_start(wout_s[:], wout_p[:])
            bout_s = pool.tile([1, 1], F32, name="bout_s")
            nc.sync.dma_start(bout_s[:], bout_p[:])
            acc = pool.tile([1, ng], F32, name="acc")
            for bi, blk in enumerate(range(0, ng, 512)):
                n = min(512, ng - blk)
                ps = rep(PS[:], (bi % 8) * 512, [[4096, 1], [1, n]])
                for k in range(nk):
                    nc.tensor.matmul(ps, wout_s[:, k:k + 1],
                                     hk_s[:, k * ng + blk:k * ng + blk + n],
                                     start=(k == 0), stop=(k == nk - 1))
                nc.vector.tensor_copy(acc[:, blk:blk + n], ps)
            nc.vector.tensor_scalar(acc[:], acc[:], bout_s[:, 0:1], None,
                                    op0=OP.add)
            nc.sync.dma_start(
                out_p[:].rearrange("(p s) o -> p (s o)", p=1), acc[:])


# ---------------------------------------------------------------- entry point

def _run(inputs, trace=False):
    x = np.asarray(inputs["x"], np.float32)
    edge_index = np.asarray(inputs["edge_index"])
    n_nodes, n_edges = x.shape[0], edge_index.shape[1]
    w = {k: np.asarray(inputs[k], np.float32) for k in
         ("W1", "a_src1", "a_dst1", "b1", "W2", "a_src2", "a_dst2", "b2",
          "W_out", "b_out")}

    src_all = np.concatenate([edge_index[0], np.arange(n_nodes)]).astype(np.int64)
    dst_all = np.concatenate([edge_index[1], np.arange(n_nodes)]).astype(np.int64)

    for bump in range(3):
        cfg = make_cfg(n_nodes, n_edges, bump=bump)
        try:
            core_of = dst_all // cfg.npc
            srt = np.argsort(core_of, kind="stable")
            ss, dd = src_all[srt], dst_all[srt]
            bounds = np.searchsorted(core_of[srt], np.arange(NCORES + 1))
            per = [_prep_core(ss[bounds[c]:bounds[c + 1]],
                              dd[bounds[c]:bounds[c + 1]] - c * cfg.npc, cfg)
                   for c in range(NCORES)]
            break
        except OverflowError:
            continue
    else:
        raise RuntimeError("could not pack edges")

    npc, ct, nw = cfg.npc, cfg.ct, cfg.nw
    nstg = cfg.nsub * SUB_CAP
    results = []

    # ---- A1: per-node attention logits for layer 1
    xb = x.astype(NP_BF16)
    v12 = np.stack([w["W1"] @ w["a_src1"], w["W1"] @ w["a_dst1"]],
                   axis=1).astype(NP_BF16)
    nc = bacc.Bacc(num_devices=NCORES)
    _build_node_al(nc, cfg)
    nc.compile()
    maps = [{"xfm": np.ascontiguousarray(xb[c * npc:(c + 1) * npc].T),
             "v12": v12} for c in range(NCORES)]
    res = run_bass_kernel_spmd(nc, maps, list(range(NCORES)), trace=trace)
    results.append(res)
    # alsd[c] is [128, nt*2]; node t*128+p -> cols 2t+k
    als1_g = np.empty(n_nodes, np.float32)
    ald1_g = np.empty(n_nodes, np.float32)
    for c in range(NCORES):
        a = np.asarray(res.results[c]["alsd"]).reshape(128, npc // 128, 2)
        als1_g[c * npc:(c + 1) * npc] = a[:, :, 0].T.reshape(npc)
        ald1_g[c * npc:(c + 1) * npc] = a[:, :, 1].T.reshape(npc)

    iota_rep = np.tile(np.arange(SUB_CAP, dtype=np.float32)[None, :, None],
                       (128, 1, WCH)).reshape(128, SUB_CAP * WCH).astype(NP_BF16)

    def wext_of(W, b):
        we = np.zeros((128, XW), np.float32)
        we[:F, :F] = W
        we[F, :F] = b
        we[F, F] = 1.0
        return we.astype(NP_BF16)

    def unswizzle_stg(raw):
        # raw [128, nw*256] -> [nstg, F]: slot (w*4+g)*128+p
        r = raw.reshape(128, nw * 4, F)
        return r.transpose(1, 0, 2).reshape(nstg, F)

    def gat_launch(feat_b16, als_g, ald_g, wext, extra, first):
        nc = bacc.Bacc(num_devices=NCORES)
        _build_gat(nc, cfg, first=first)
        nc.compile()
        maps = []
        shared = {"wext": wext, "iotar": iota_rep, **extra}
        for c in range(NCORES):
            slot_src, slot_dst, seg_slin, _ = per[c]
            xe3 = np.empty((ct, 128, XW), NP_BF16)
            xe3[:, :, :F] = feat_b16[slot_src].reshape(ct, 128, F)
            xe3[:, :, F] = 1.0
            esc3 = np.empty((ct, 128, 2), NP_BF16)
            esc3[:, :, 0] = als_g[slot_src].reshape(ct, 128)
            esc3[:, :, 1] = ald_g[slot_dst + c * npc].reshape(ct, 128)
            m = dict(shared)
            m["xe"] = np.ascontiguousarray(
                xe3.transpose(1, 0, 2)).reshape(128, ct * XW)
            m["esc"] = np.ascontiguousarray(
                esc3.transpose(1, 0, 2)).reshape(128, ct * 2)
            m["seg"] = np.ascontiguousarray(
                seg_slin.reshape(ct, 128).T.astype(NP_BF16))
            maps.append(m)
        res = run_bass_kernel_spmd(nc, maps, list(range(NCORES)), trace=trace)
        results.append(res)
        hn = np.empty((n_nodes, F), NP_BF16)
        al2n = None
        if first:
            al2n = np.empty((2, n_nodes), np.float32)
        for c in range(NCORES):
            stg = unswizzle_stg(np.asarray(res.results[c]["stg"]))
            hn[c * npc:(c + 1) * npc] = stg[per[c][3]]
            if first:
                # al2[p, k*nw*4 + w*4+g] = al2_k[slot (w*4+g)*128+p]
                a2 = np.asarray(res.results[c]["al2"]).reshape(128, 2, nw * 4)
                a2s = a2.transpose(1, 2, 0).reshape(2, nstg)
                al2n[:, c * npc:(c + 1) * npc] = a2s[:, per[c][3]]
        return hn, al2n

    vs2 = np.tile((w["W2"] @ w["a_src2"])[None, :], (128, 1)).astype(NP_BF16)
    vd2 = np.tile((w["W2"] @ w["a_dst2"])[None, :], (128, 1)).astype(NP_BF16)
    h1, al2n = gat_launch(xb, als1_g, ald1_g, wext_of(w["W1"], w["b1"]),
                          {"vsrep": vs2, "vdrep": vd2}, first=True)
    h2, _ = gat_launch(h1, al2n[0], al2n[1], wext_of(w["W2"], w["b2"]),
                       {}, first=False)

    # ---- F: per-subgraph Linear
    nk = SUBG * F // 128
    nc = bacc.Bacc(num_devices=NCORES)
    _build_final(nc, cfg)
    nc.compile()
    woutk = np.ascontiguousarray(
        w["W_out"][:, 0].reshape(nk, 128).T).astype(NP_BF16)
    boutr = np.full((1, 1), float(w["b_out"][0]), np.float32)
    maps = []
    for c in range(NCORES):
        hg = h2[c * npc:(c + 1) * npc].reshape(cfg.ng, nk, 128)
        maps.append({"hk": np.ascontiguousarray(
            hg.transpose(2, 1, 0)).reshape(128, nk * cfg.ng),
            "woutk": woutk, "boutr": boutr})
    res = run_bass_kernel_spmd(nc, maps, list(range(NCORES)), trace=trace)
    results.append(res)
    out = np.concatenate([np.asarray(r["out"]) for r in res.results], axis=0)
    return out.astype(np.float32), results


def kernel(**inputs) -> np.ndarray:
    out, _ = _run(inputs, trace=False)
    return out


# revision 14
# speedup vs baseline: 5.5674x; 5.5674x over previous
"""Distributed 2-layer GATConv + per-subgraph Linear on 8 TRN2 NeuronCores.

Key identity: out_dst = (sum_e ex_e * x_src_e) @ W / sum_e ex_e + b, i.e. the
dense transform W is applied AFTER aggregation (per node), and the attention
logits al_s/al_d are per-node scalars x . (W @ a). So per edge we only ship
the raw src feature row plus two scalars; no per-edge transform matmul.

Launches:
  A1: per-node al_s1/al_d1 = x . (W1@a_src1), x . (W1@a_dst1)   (tiny)
  C1: layer-1 edge aggregation + W1/bias/relu epilogue + per-node
      al_s2/al_d2 for layer 2 (fused)
  C2: layer-2 edge aggregation + W2/bias/relu epilogue
  F : per-subgraph Linear as a PE matvec

Edge layout: dst-sharded edges, sorted by dst, bin-packed into 32-dst
subtiles of 256 slots (2 chunks of 128). Per chunk one PE matmul:
lhsT = [x_src | 1] (128 edges x 65, stationary), rhs = onehot(dstcol)*ex
(128 x 32) accumulating [sum ex*x | sum ex] per dst column into PSUM.
A second small matmul per 4 subtiles applies Wext = [[W,0],[b,1]] giving
[num@W + denom*b | denom] per dst; epilogue multiplies by 1/denom (relu'd).
Device outputs stay in SBUF-native [128, cols] layouts (large DMA
descriptors); the host unswizzles by precomputed index maps. The host
performs only data movement; model arithmetic runs on device.
"""

import dataclasses
import math
import numpy as np

import concourse.bass as bass
import concourse.bacc as bacc
import concourse.mybir as mybir
import concourse.tile as tile
from concourse.bass_utils import run_bass_kernel_spmd

F32 = mybir.dt.float32
BF16 = mybir.dt.bfloat16
NP_BF16 = mybir.dt.np(BF16)

NCORES = 8
F = 64
SUBG = 20
XW = 65           # src feature cols + ones col
SUB_SLOTS = 256   # slots per subtile (2 chunks)
SUB_CAP = 16
NEG_SLOPE = 0.2
WCH = 64          # chunks per window


@dataclasses.dataclass
class Cfg:
    n: int
    npc: int
    nsub: int
    e_pad: int
    ct: int
    nw: int
    ng: int


def make_cfg(n_nodes: int, n_edges: int, bump: int = 0) -> Cfg:
    npc = n_nodes // NCORES
    epc = (n_edges + n_nodes) / NCORES
    slots = epc * 1.09 + 2 * SUB_SLOTS
    nsub = math.ceil(slots / SUB_SLOTS / (2 * WCH // 2)) * (WCH // 2) * 2
    nsub += (WCH // 2) * 2 * bump
    e_pad = nsub * SUB_SLOTS
    ct = e_pad // 128
    return Cfg(n=n_nodes, npc=npc, nsub=nsub, e_pad=e_pad,
               ct=ct, nw=ct // WCH, ng=npc // SUBG)


# ---------------------------------------------------------------- host prep

def _pack_bins(deg, cfg):
    """Largest-fit packing of dsts into bins of 256 slots / 32 dsts."""
    npc = deg.shape[0]
    vmax = int(deg.max())
    assert vmax <= SUB_SLOTS
    order = np.argsort(deg, kind="stable")
    bucket_start = np.searchsorted(deg[order], np.arange(vmax + 2))
    ptr = bucket_start[:-1].copy()
    cnt = (bucket_start[1:] - bucket_start[:-1]).copy()
    sub_of = np.empty(npc, np.int32)
    col_of = np.empty(npc, np.int32)
    remaining = npc
    b = 0
    while remaining > 0:
        rem = SUB_SLOTS
        col = 0
        while col < SUB_CAP:
            v = min(rem, vmax)
            while v >= 0 and cnt[v] == 0:
                v -= 1
            if v < 0:
                break
            d = order[ptr[v]]
            ptr[v] += 1
            cnt[v] -= 1
            sub_of[d] = b
            col_of[d] = col
            col += 1
            rem -= v
            remaining -= 1
        b += 1
        if b > cfg.nsub:
            raise OverflowError("bin packing exceeded subtile budget")
    return sub_of, col_of


def _prep_core(src, dst_local, cfg):
    """Per-core slot layout. Returns (slot_src, slot_dst, seg_slin,
    stg_of_node)."""
    npc, e_pad = cfg.npc, cfg.e_pad
    deg = np.bincount(dst_local, minlength=npc)
    sub_of, col_of = _pack_bins(deg, cfg)

    order = np.argsort(dst_local, kind="stable")
    src_bd = src[order]
    starts = np.zeros(npc + 1, np.int64)
    np.cumsum(deg, out=starts[1:])

    binkey = sub_of.astype(np.int64) * SUB_CAP + col_of
    dorder = np.argsort(binkey, kind="stable")
    deg_bo = deg[dorder]
    csum = np.cumsum(deg_bo)
    bin_ids = sub_of[dorder]
    nb = int(bin_ids.max()) + 1
    first_of_bin = np.searchsorted(bin_ids, np.arange(nb), side="left")
    base_cum = np.where(first_of_bin > 0, csum[np.maximum(first_of_bin - 1, 0)], 0)
    off_in_bin = np.zeros(npc, np.int64)
    off_in_bin[dorder] = (csum - deg_bo) - base_cum[bin_ids]
    slot0 = sub_of.astype(np.int64) * SUB_SLOTS + off_in_bin

    slot_src = np.zeros(e_pad, np.int64)
    slot_dst = np.zeros(e_pad, np.int64)
    seg_slin = np.full(e_pad, 64, np.int64)

    edst = dst_local[order]
    within = np.arange(len(order), dtype=np.int64) - starts[edst]
    eslot = slot0[edst] + within
    slot_src[eslot] = src_bd
    slot_dst[eslot] = edst
    seg_slin[eslot] = col_of[edst]
    stg_of_node = sub_of.astype(np.int64) * SUB_CAP + col_of
    return slot_src, slot_dst, seg_slin, stg_of_node


def bc(ap, ins_idx, pair):
    aps = list(ap.ap)
    aps.insert(ins_idx, list(pair))
    return dataclasses.replace(ap, ap=aps)


def rep(ap, offset, new_ap):
    return dataclasses.replace(ap, offset=ap.offset + offset, ap=new_ap)


# ---------------------------------------------------------------- A1 launch

def _build_node_al(nc, cfg):
    npc = cfg.npc
    nt = npc // 128
    xfm_p = nc.declare_dram_parameter("xfm", [F, npc], BF16, isOutput=False)
    v12_p = nc.declare_dram_parameter("v12", [F, 2], BF16, isOutput=False)
    # SBUF-native: alsd[p, 2t+k] = al_{s,d}[node t*128+p]
    out_p = nc.declare_dram_parameter("alsd", [128, nt * 2], F32,
                                      isOutput=True)
    with nc.psum_tensor([128, 4096], F32) as PS, tile.TileContext(nc) as tc:
        with tc.tile_pool(name="p", bufs=1) as pool:
            xfm_s = pool.tile([F, npc], BF16, name="xfm_s")
            nc.sync.dma_start(xfm_s[:], xfm_p[:])
            v12_s = pool.tile([F, 2], BF16, name="v12_s")
            nc.sync.dma_start(v12_s[:], v12_p[:])
            acc = pool.tile([128, nt * 2], F32, name="acc")
            for t in range(nt):
                bank = (t // 256) % 8
                ps = rep(PS[:], bank * 512 + (t % 256) * 2,
                         [[4096, 128], [1, 2]])
                nc.tensor.matmul(ps, xfm_s[:, t * 128:(t + 1) * 128],
                                 v12_s[:], start=True, stop=True)
            nbank = (nt + 255) // 256
            for b in range(nbank):
                n = min(256, nt - b * 256)
                psb = rep(PS[:], (b % 8) * 512, [[4096, 128], [1, 2 * n]])
                nc.vector.tensor_copy(acc[:, b * 512:b * 512 + 2 * n], psb)
            nc.sync.dma_start(out_p[:], acc[:])


# ---------------------------------------------------------------- GAT launch

def _build_gat(nc, cfg, first):
    AT = mybir.ActivationFunctionType
    OP = mybir.AluOpType
    ct, nw = cfg.ct, cfg.nw

    xe_p = nc.declare_dram_parameter("xe", [128, ct * XW], BF16, isOutput=False)
    esc_p = nc.declare_dram_parameter("esc", [128, ct * 2], BF16,
                                      isOutput=False)
    seg_p = nc.declare_dram_parameter("seg", [128, ct], BF16, isOutput=False)
    wext_p = nc.declare_dram_parameter("wext", [128, XW], BF16, isOutput=False)
    iota_p = nc.declare_dram_parameter("iotar", [128, SUB_CAP * WCH], BF16,
                                       isOutput=False)
    # SBUF-native: stg[p, (w*4+g)*64 + f] = h[slot (w*4+g)*128+p, f]
    stg_p = nc.declare_dram_parameter("stg", [128, nw * 256], BF16,
                                      isOutput=True)
    if first:
        vs_p = nc.declare_dram_parameter("vsrep", [128, F], BF16,
                                         isOutput=False)
        vd_p = nc.declare_dram_parameter("vdrep", [128, F], BF16,
                                         isOutput=False)
        # al2[p, k*nw*4 + w*4 + g] = al2_k[slot (w*4+g)*128+p]
        al2_p = nc.declare_dram_parameter("al2", [128, 2 * nw * 4], F32,
                                          isOutput=True)

    with nc.psum_tensor([128, 4096], F32) as PS, tile.TileContext(nc) as tc:
        with (
            tc.tile_pool(name="const", bufs=1) as cpool,
            tc.tile_pool(name="xw", bufs=2) as xpool,
            tc.tile_pool(name="esc", bufs=2) as epool,
            tc.tile_pool(name="sw", bufs=2) as swpool,
            tc.tile_pool(name="u", bufs=2) as upool,
            tc.tile_pool(name="stage", bufs=2) as stpool,
            tc.tile_pool(name="small", bufs=3) as smpool,
        ):
            wext_s = cpool.tile([128, XW], BF16, name="wext_s")
            nc.sync.dma_start(wext_s[:], wext_p[:])
            iota_s = cpool.tile([128, SUB_CAP * WCH], BF16, name="iota_s")
            nc.sync.dma_start(iota_s[:], iota_p[:])
            if first:
                vs_s = cpool.tile([128, F], BF16, name="vs_s")
                nc.sync.dma_start(vs_s[:], vs_p[:])
                vd_s = cpool.tile([128, F], BF16, name="vd_s")
                nc.sync.dma_start(vd_s[:], vd_p[:])
                al2acc = cpool.tile([128, 2 * nw * 4], F32, name="al2acc")

            pend = None

            def emit_tail(pw, u_prev):
                base2 = (2 + (pw % 2)) * 512
                for g in range(4):
                    ps2 = rep(PS[:], base2 + g * 128, [[4096, 128], [1, XW]])
                    nc.tensor.matmul(ps2, u_prev[:, g * 128:(g + 1) * 128],
                                     wext_s[0:XW, :], start=True, stop=True)
                dsb = smpool.tile([128, 4], F32, name="dsb", tag="dsb")
                nc.vector.tensor_scalar(
                    dsb[:], rep(PS[:], base2 + F, [[4096, 128], [128, 4]]),
                    1e-20, None, op0=OP.add)
                rd = smpool.tile([128, 4], F32, name="rd", tag="rd")
                nc.vector.reciprocal(rd[:], dsb[:])
                stg_t = stpool.tile([128, 4 * F], BF16, name="stg", tag="stg")
                stg3 = stg_t[:].rearrange("p (g e) -> p g e", e=F)
                ps2num = rep(PS[:], base2, [[4096, 128], [128, 4], [1, F]])
                nc.vector.tensor_tensor(stg3, ps2num, bc(rd[:], 2, (0, F)),
                                        op=OP.mult)
                nc.vector.tensor_scalar(stg_t[:], stg_t[:], 0.0, None,
                                        op0=OP.max)
                nc.sync.dma_start(stg_p[:, pw * 256:(pw + 1) * 256], stg_t[:])
                if first:
                    tmp = stpool.tile([128, 4 * F], BF16, name="tmp",
                                      tag="tmp")
                    tmp3 = tmp[:].rearrange("p (g e) -> p g e", e=F)
                    nc.gpsimd.tensor_tensor(tmp3, stg3, bc(vs_s[:], 1, (0, 4)),
                                            op=OP.mult)
                    o0 = rep(al2acc[:], pw * 4, [[2 * nw * 4, 128], [1, 4]])
                    nc.vector.tensor_reduce(o0, tmp3, mybir.AxisListType.X,
                                            OP.add)
                    tmp2 = stpool.tile([128, 4 * F], BF16, name="tmp2",
                                       tag="tmp2")
                    tmp23 = tmp2[:].rearrange("p (g e) -> p g e", e=F)
                    nc.gpsimd.tensor_tensor(tmp23, stg3,
                                            bc(vd_s[:], 1, (0, 4)),
                                            op=OP.mult)
                    o1 = rep(al2acc[:], nw * 4 + pw * 4,
                             [[2 * nw * 4, 128], [1, 4]])
                    nc.vector.tensor_reduce(o1, tmp23, mybir.AxisListType.X,
                                            OP.add)

            for w in range(nw):
                xw_t = xpool.tile([128, WCH * XW], BF16, name="xw", tag="xw")
                nc.sync.dma_start(xw_t[:], xe_p[:, w * WCH * XW:
                                                (w + 1) * WCH * XW])
                esc_t = epool.tile([128, WCH * 2], BF16, name="esc", tag="esc")
                nc.sync.dma_start(esc_t[:], esc_p[:, w * WCH * 2:
                                                  (w + 1) * WCH * 2])
                seg_t = epool.tile([128, WCH], BF16, name="seg", tag="seg")
                nc.sync.dma_start(seg_t[:], seg_p[:, w * WCH:(w + 1) * WCH])
                als_v = rep(esc_t[:], 0, [[WCH * 2, 128], [2, WCH]])
                ald_v = rep(esc_t[:], 1, [[WCH * 2, 128], [2, WCH]])

                e_t = smpool.tile([128, WCH], F32, name="e", tag="e")
                nc.vector.tensor_tensor(e_t[:], als_v, ald_v, op=OP.add)
                lr_t = smpool.tile([128, WCH], F32, name="lr", tag="lr")
                nc.vector.tensor_scalar(lr_t[:], e_t[:], NEG_SLOPE, None,
                                        op0=OP.mult)
                nc.vector.tensor_tensor(lr_t[:], lr_t[:], e_t[:], op=OP.max)
                exb = smpool.tile([128, WCH], BF16, name="exb", tag="exb")
                nc.scalar.activation(exb[:], lr_t[:], AT.Exp)

                # Sw[p, d, c]: dst-col-major so every operand is packed
                SC = SUB_CAP
                Sw_t = swpool.tile([128, SC * WCH], BF16, name="Sw", tag="Sw")
                sw3 = rep(Sw_t[:], 0, [[SC * WCH, 128], [WCH, SC], [1, WCH]])
                io3 = rep(iota_s[:], 0, [[SC * WCH, 128], [WCH, SC], [1, WCH]])
                nc.vector.tensor_tensor(sw3, io3, bc(seg_t[:], 1, (0, SC)),
                                        op=OP.is_equal)
                nc.vector.tensor_tensor(sw3, sw3, bc(exb[:], 1, (0, SC)),
                                        op=OP.mult)

                u_t = upool.tile([XW, 512], BF16, name="u", tag="u")
                for cl in range(WCH):
                    sb, k = divmod(cl, 2)
                    bank = w % 2
                    psu = rep(PS[:], bank * 512 + sb * SC,
                              [[4096, XW], [1, SC]])
                    swv = rep(Sw_t[:], cl, [[SC * WCH, 128], [WCH, SC]])
                    nc.tensor.matmul(psu, xw_t[:, cl * XW:(cl + 1) * XW],
                                     swv, start=(k == 0), stop=(k == 1))
                    if cl == 15 and pend is not None:
                        emit_tail(*pend)
                        pend = None
                    if cl in (15, 31, 47, 63):
                        q = cl // 16
                        psb = rep(PS[:], bank * 512 + q * 128,
                                  [[4096, XW], [1, 128]])
                        nc.scalar.activation(u_t[:, q * 128:(q + 1) * 128],
                                             psb, AT.Copy)
                        if cl == 63:
                            pend = (w, u_t)
            emit_tail(*pend)
            if first:
                nc.sync.dma_start(al2_p[:], al2acc[:])


# ---------------------------------------------------------------- final linear

def _build_final(nc, cfg):
    OP = mybir.AluOpType
    ng = cfg.ng
    nk = SUBG * F // 128  # 10
    hk_p = nc.declare_dram_parameter("hk", [128, nk * ng], BF16,
                                     isOutput=False)
    wout_p = nc.declare_dram_parameter("woutk", [128, nk], BF16,
                                       isOutput=False)
    bout_p = nc.declare_dram_parameter("boutr", [1, 1], F32, isOutput=False)
    out_p = nc.declare_dram_parameter("out", [ng, 1], F32, isOutput=True)
    with nc.psum_tensor([128, 4096], F32) as PS, tile.TileContext(nc) as tc:
        with tc.tile_pool(name="p", bufs=1) as pool:
            hk_s = pool.tile([128, nk * ng], BF16, name="hk_s")
            nc.sync.dma_start(hk_s[:], hk_p[:])
            wout_s = pool.tile([128, nk], BF16, name="wout_s")
            nc.sync.dma_start(wout_s[:], wout_p[:])
            bout_s = pool.tile([1, 1], F32, name="bout_s")
            nc.sync.dma_start(bout_s[:], bout_p[:])
            acc = pool.tile([1, ng], F32, name="acc")
            for bi, blk in enumerate(range(0, ng, 512)):
                n = min(512, ng - blk)
                ps = rep(PS[:], (bi % 8) * 512, [[4096, 1], [1, n]])
                for k in range(nk):
                    nc.tensor.matmul(ps, wout_s[:, k:k + 1],
                                     hk_s[:, k * ng + blk:k * ng + blk + n],
                                     start=(k == 0), stop=(k == nk - 1))
                nc.vector.tensor_copy(acc[:, blk:blk + n], ps)
            nc.vector.tensor_scalar(acc[:], acc[:], bout_s[:, 0:1], None,
                                    op0=OP.add)
            nc.sync.dma_start(
                out_p[:].rearrange("(p s) o -> p (s o)", p=1), acc[:])


# ---------------------------------------------------------------- entry point

def _run(inputs, trace=False):
    x = np.asarray(inputs["x"], np.float32)
    edge_index = np.asarray(inputs["edge_index"])
    n_nodes, n_edges = x.shape[0], edge_index.shape[1]
    w = {k: np.asarray(inputs[k], np.float32) for k in
         ("W1", "a_src1", "a_dst1", "b1", "W2", "a_src2", "a_dst2", "b2",
          "W_out", "b_out")}

    src_all = np.concatenate([edge_index[0], np.arange(n_nodes)]).astype(np.int64)
    dst_all = np.concatenate([edge_index[1], np.arange(n_nodes)]).astype(np.int64)

    for bump in range(3):
        cfg = make_cfg(n_nodes, n_edges, bump=bump)
        try:
            core_of = dst_all // cfg.npc
            srt = np.argsort(core_of, kind="stable")
            ss, dd = src_all[srt], dst_all[srt]
            bounds = np.searchsorted(core_of[srt], np.arange(NCORES + 1))
            per = [_prep_core(ss[bounds[c]:bounds[c + 1]],
                              dd[bounds[c]:bounds[c + 1]] - c * cfg.npc, cfg)
                   for c in range(NCORES)]
            break
        except OverflowError:
            continue
    else:
        raise RuntimeError("could not pack edges")

    npc, ct, nw = cfg.npc, cfg.ct, cfg.nw
    nstg = cfg.nsub * SUB_CAP
    results = []

    # ---- A1: per-node attention logits for layer 1
    xb = x.astype(NP_BF16)
    v12 = np.stack([w["W1"] @ w["a_src1"], w["W1"] @ w["a_dst1"]],
                   axis=1).astype(NP_BF16)
    nc = bacc.Bacc(num_devices=NCORES)
    _build_node_al(nc, cfg)
    nc.compile()
    maps = [{"xfm": np.ascontiguousarray(xb[c * npc:(c + 1) * npc].T),
             "v12": v12} for c in range(NCORES)]
    res = run_bass_kernel_spmd(nc, maps, list(range(NCORES)), trace=trace)
    results.append(res)
    # alsd[c] is [128, nt*2]; node t*128+p -> cols 2t+k
    als1_g = np.empty(n_nodes, np.float32)
    ald1_g = np.empty(n_nodes, np.float32)
    for c in range(NCORES):
        a = np.asarray(res.results[c]["alsd"]).reshape(128, npc // 128, 2)
        als1_g[c * npc:(c + 1) * npc] = a[:, :, 0].T.reshape(npc)
        ald1_g[c * npc:(c + 1) * npc] = a[:, :, 1].T.reshape(npc)

    iota_rep = np.tile(np.arange(SUB_CAP, dtype=np.float32)[None, :, None],
                       (128, 1, WCH)).reshape(128, SUB_CAP * WCH).astype(NP_BF16)

    def wext_of(W, b):
        we = np.zeros((128, XW), np.float32)
        we[:F, :F] = W
        we[F, :F] = b
        we[F, F] = 1.0
        return we.astype(NP_BF16)

    def unswizzle_stg(raw):
        # raw [128, nw*256] -> [nstg, F]: slot (w*4+g)*128+p
        r = raw.reshape(128, nw * 4, F)
        return r.transpose(1, 0, 2).reshape(nstg, F)

    def gat_launch(feat_b16, als_g, ald_g, wext, extra, first):
        nc = bacc.Bacc(num_devices=NCORES)
        _build_gat(nc, cfg, first=first)
        nc.compile()
        maps = []
        shared = {"wext": wext, "iotar": iota_rep, **extra}
        for c in range(NCORES):
            slot_src, slot_dst, seg_slin, _ = per[c]
            xe3 = np.empty((ct, 128, XW), NP_BF16)
            xe3[:, :, :F] = feat_b16[slot_src].reshape(ct, 128, F)
            xe3[:, :, F] = 1.0
            esc3 = np.empty((ct, 128, 2), NP_BF16)
            esc3[:, :, 0] = als_g[slot_src].reshape(ct, 128)
            esc3[:, :, 1] = ald_g[slot_dst + c * npc].reshape(ct, 128)
            m = dict(shared)
            m["xe"] = np.ascontiguousarray(
                xe3.transpose(1, 0, 2)).reshape(128, ct * XW)
            m["esc"] = np.ascontiguousarray(
                esc3.transpose(1, 0, 2)).reshape(128, ct * 2)
            m["seg"] = np.ascontiguousarray(
                seg_slin.reshape(ct, 128).T.astype(NP_BF16))
            maps.append(m)
        res = run_bass_kernel_spmd(nc, maps, list(range(NCORES)), trace=trace)
        results.append(res)
        hn = np.empty((n_nodes, F), NP_BF16)
        al2n = None
        if first:
            al2n = np.empty((2, n_nodes), np.float32)
        for c in range(NCORES):
            stg = unswizzle_stg(np.asarray(res.results[c]["stg"]))
            hn[c * npc:(c + 1) * npc] = stg[per[c][3]]
            if first:
                # al2[p, k*nw*4 + w*4+g] = al2_k[slot (w*4+g)*128+p]
                a2 = np.asarray(res.results[c]["al2"]).reshape(128, 2, nw * 4)
                a2s = a2.transpose(1, 2, 0).reshape(2, nstg)
                al2n[:, c * npc:(c + 1) * npc] = a2s[:, per[c][3]]
        return hn, al2n

    vs2 = np.tile((w["W2"] @ w["a_src2"])[None, :], (128, 1)).astype(NP_BF16)
    vd2 = np.tile((w["W2"] @ w["a_dst2"])[None, :], (128, 1)).astype(NP_BF16)
    h1, al2n = gat_launch(xb, als1_g, ald1_g, wext_of(w["W1"], w["b1"]),
                          {"vsrep": vs2, "vdrep": vd2}, first=True)
    h2, _ = gat_launch(h1, al2n[0], al2n[1], wext_of(w["W2"], w["b2"]),
                       {}, first=False)

    # ---- F: per-subgraph Linear
    nk = SUBG * F // 128
    nc = bacc.Bacc(num_devices=NCORES)
    _build_final(nc, cfg)
    nc.compile()
    woutk = np.ascontiguousarray(
        w["W_out"][:, 0].reshape(nk, 128).T).astype(NP_BF16)
    boutr = np.full((1, 1), float(w["b_out"][0]), np.float32)
    maps = []
    for c in range(NCORES):
        hg = h2[c * npc:(c + 1) * npc].reshape(cfg.ng, nk, 128)
        maps.append({"hk": np.ascontiguousarray(
            hg.transpose(2, 1, 0)).reshape(128, nk * cfg.ng),
            "woutk": woutk, "boutr": boutr})
    res = run_bass_kernel_spmd(nc, maps, list(range(NCORES)), trace=trace)
    results.append(res)
    out = np.concatenate([np.asarray(r["out"]) for r in res.results], axis=0)
    return out.astype(np.float32), results


def kernel(**inputs) -> np.ndarray:
    out, _ = _run(inputs, trace=False)
    return out
